# revision 42
# baseline (speedup 1.0000x reference)
"""Trainium2 Bass kernel for nn_Attention_Module (sparse_attention).

Computation per batch b (x_b: [C=256, T=4096] fp32):
    energy = x_b @ x_b^T                      # (256, 256), K=4096
    attn   = softmax(rowmax(energy) - energy) # == exp(mu - e)/Z, mu = rowmin
    out    = gamma * (attn @ x_b) + x_b

Strategy (8 cores, pure data-parallel, 4 batches/core):
  - x is loaded from HBM exactly ONCE, as fp16 in t-major layout
    (xt[b, p, k, c] = x[b, c, k*128+p]) with 8KB/partition DMA lines
    (1 descriptor per partition -> cheap HWDGE issue). The c-major copy
    needed by matmul2 (xn) is derived on-chip with PE transposes.
    DMA drops from 41.9 MB/core (baseline) to 25.2 MB/core.
  - Energy is symmetric: only blocks 00/01/11 are computed; block 10 is
    a PE transpose of block 01 (saves 1/4 of matmul1).
  - matmul1 (both row-block chains) and the xn transposes are
    interleaved per k-tile so the PE tracks DMA arrival during the
    pipeline fill instead of stalling on whole tiles.
  - B = gamma*diag(1/Z)*P + I is materialized directly as the matmul2
    weights: P rows are scaled by 1/Z before the PE transpose, gamma is
    applied during the PSUM->SBUF copy of P^T, and the +x residual is
    the exact identity diagonal. matmul2 yields final output values, so
    PSUM->SBUF drains are plain copies (round-robined ACT/DVE).
  - Software pipeline: slot b = loads(b+1), mm1+xnT(b), mm2(b-1)
    (hides b's softmax latency), then softmax/At(b).
"""

import numpy as np

B, C, T = 32, 256, 4096
NCORES = 8
NB = B // NCORES  # batches per core
P = 128
KT = T // P  # 32 t-tiles of 128
KH = KT // 2  # 16 per half (xn is built as two half-tiles)
TC = T // 512  # 8 output chunks per m-block

_CACHE = {}


def _build_nc(variant=None):
    variant = variant or {}
    from contextlib import ExitStack

    import concourse.bacc as bacc
    import concourse.bass as bass
    import concourse.tile as tile
    from concourse import mybir

    f32 = mybir.dt.float32
    f16 = mybir.dt.float16
    ts = bass.ts

    nc = bacc.Bacc(
        "TRN2",
        target_bir_lowering=False,
        debug=False,
        enable_asserts=False,
        num_devices=NCORES,
    )

    # xt[b, p, k, c] = x[b, c, k*128+p]; per-partition lines are 8KB.
    xt_h = nc.dram_tensor("xt", [NB, P, KT, C], f16, kind="ExternalInput")
    # xnh[b, k2, p, t'] = x[b, k2*128+p, 2048+t']: upper-t half of the
    # c-major copy, loaded directly (mid-slot, behind the critical loads)
    # so only the lower half of xn needs PE transposes.
    xnh_h = nc.dram_tensor("xnh", [NB, 2, P, T // 2], f16, kind="ExternalInput")
    # aux: col0 = gamma (f32), cols 4:132 = identity (f32)
    aux_h = nc.dram_tensor("aux", [P, 132], f32, kind="ExternalInput")
    idn_h = nc.dram_tensor("idn", [P, P], f16, kind="ExternalInput")
    o_h = nc.dram_tensor("o", [NB, C, T], f32, kind="ExternalOutput")

    with tile.TileContext(nc) as tc:
        with ExitStack() as ctx:
            singles = ctx.enter_context(tc.tile_pool(name="singles", bufs=1))
            xt_pool = ctx.enter_context(tc.tile_pool(name="xt", bufs=2))
            xt1_pool = ctx.enter_context(tc.tile_pool(name="xt1", bufs=1))
            xn_pool = ctx.enter_context(tc.tile_pool(name="xn", bufs=3))
            out_pool = ctx.enter_context(tc.tile_pool(name="out", bufs=3))
            att_pool = ctx.enter_context(tc.tile_pool(name="att", bufs=2))
            small = ctx.enter_context(tc.tile_pool(name="small", bufs=3))
            psum_e = ctx.enter_context(
                tc.tile_pool(name="psum_e", bufs=1, space="PSUM")
            )
            psum_x = ctx.enter_context(
                tc.tile_pool(name="psum_x", bufs=2, space="PSUM")
            )
            psum_o = ctx.enter_context(
                tc.tile_pool(name="psum_o", bufs=4, space="PSUM")
            )

            xt_ap = xt_h.ap()
            xnh_ap = xnh_h.ap()
            o_ap = o_h.ap()

            aux = singles.tile([P, 132], f32)
            nc.scalar.dma_start(aux[:], aux_h.ap())
            idn = singles.tile([P, P], f16)
            nc.scalar.dma_start(idn[:], idn_h.ap())
            gv = aux[:, 0:1]
            idn32 = aux[:, 4:132]

            def issue_loads(b):
                if b == 0:
                    # eight separate tiles: mm1 starts after 256KB lands,
                    # and arrivals pace the PE through the whole fill
                    KQ = KT // 8
                    qs = []
                    for q in range(8):
                        t_ = xt_pool.tile(
                            [P, KQ, C], f16, tag=f"xq{q}", name=f"xq{q}"
                        )
                        nc.sync.dma_start(
                            t_[:], xt_ap[b, :, q * KQ : (q + 1) * KQ, :]
                        )
                        qs.append(t_)
                    return (qs, KT // 8)
                if b == 1:
                    # four single-use quarter tiles (own bufs=1 pool: no
                    # cross-batch tag reuse, so no recycle anti-deps) pace
                    # mm1(1) through the congested slot-0 window
                    KQ = KT // 4
                    qs = []
                    for q in range(4):
                        t_ = xt1_pool.tile(
                            [P, KQ, C], f16, tag=f"xr{q}", name=f"xr{q}"
                        )
                        nc.sync.dma_start(
                            t_[:], xt_ap[b, :, q * KQ : (q + 1) * KQ, :]
                        )
                        qs.append(t_)
                    return (qs, KQ)
                xta = xt_pool.tile([P, KH, C], f16, tag="xta", name="xta")
                xtb = xt_pool.tile([P, KH, C], f16, tag="xtb", name="xtb")
                nc.sync.dma_start(xta[:], xt_ap[b, :, :KH, :])
                nc.sync.dma_start(xtb[:], xt_ap[b, :, KH:, :])
                return ([xta, xtb], KH)

            # round-robin copy engines for PSUM->SBUF drains
            cp_engines = [
                lambda o, i: nc.scalar.copy(o, i),
                lambda o, i: nc.vector.tensor_copy(o, i),
            ]
            cp_idx = [0]

            def copy_eng():
                e = cp_engines[cp_idx[0] % len(cp_engines)]
                cp_idx[0] += 1
                return e

            def run_mm2_m(pb, pAt, pxn, m, nsplit):
                """out(pb) row-block m = B^T @ x (final values) + stores."""
                TH = T // 2
                ot = out_pool.tile([P, T], f32, tag="ot", name="ot")
                for t8 in range(TC):  # [P,512] chunks, 4-deep psum ring
                    po = psum_o.tile([P, 512], f32)
                    tf = t8 * 512 % TH
                    for k in range(2):
                        nc.tensor.matmul(
                            po[:],
                            lhsT=pAt[m][:, k, :],
                            rhs=pxn[t8 // 4][:, k, tf : tf + 512],
                            start=(k == 0),
                            stop=(k == 1),
                        )
                    copy_eng()(ot[:, ts(t8, 512)], po[:])
                for sh in range(nsplit):
                    nc.sync.dma_start(
                        o_ap[pb].rearrange("(m p) t -> p m t", p=P)[
                            :, m, ts(sh, T // nsplit)
                        ],
                        ot[:, ts(sh, T // nsplit)],
                    )

            def run_mm2(pb, pAt, pxn):
                for m in range(2):
                    run_mm2_m(pb, pAt, pxn, m, 2)

            tiles = {0: issue_loads(0)}
            pending = None  # (b, At, xn) awaiting matmul2

            for b in range(NB):
                xtiles, kdiv = tiles.pop(b)
                if b + 1 < NB:
                    tiles[b + 1] = issue_loads(b + 1)

                xn = [
                    xn_pool.tile([P, 2, T // 2], f16, tag="xna", name="xna"),
                    xn_pool.tile([P, 2, T // 2], f16, tag="xnb", name="xnb"),
                ]
                nc.sync.dma_start(
                    xn[1][:], xnh_ap[b].rearrange("k p t -> p k t")
                )
                At = [
                    att_pool.tile([P, 2, P], f16, tag="Ata", name="Ata"),
                    att_pool.tile([P, 2, P], f16, tag="Atb", name="Atb"),
                ]
                Zs = small.tile([P, 2], f32, tag="Zs")
                rZ = small.tile([P, 2], f32, tag="rZ")

                # ---- interleaved mm1 (blocks 00/01 + block 11) and xn
                # transposes, per k-tile, tracking DMA arrival ----
                pe1 = psum_e.tile([P, C], f32, tag="pe1", name="pe1")
                pe2 = psum_e.tile([P, C], f32, tag="pe2", name="pe2")
                for k in range(KT):
                    src = xtiles[k // kdiv]
                    kk = k % kdiv
                    nc.tensor.matmul(
                        pe1[:],
                        lhsT=src[:, kk, ts(0, P)],
                        rhs=src[:, kk, :],
                        start=(k == 0),
                        stop=(k == KT - 1),
                    )
                    nc.tensor.matmul(
                        pe2[:, ts(1, P)],
                        lhsT=src[:, kk, ts(1, P)],
                        rhs=src[:, kk, ts(1, P)],
                        start=(k == 0),
                        stop=(k == KT - 1),
                    )
                    if k % 4 == 3 and k < KH:
                        g4 = k - 3
                        half = 0
                        tb = g4 * P
                        for cb in range(2):
                            px = psum_x.tile([P, 512], f16, tag="px", name="px")
                            for j in range(4):
                                kg = g4 + j
                                nc.tensor.transpose(
                                    px[:, ts(j, P)],
                                    xtiles[kg // kdiv][:, kg % kdiv, ts(cb, P)],
                                    idn[:],
                                )
                            copy_eng()(xn[half][:, cb, tb : tb + 512], px[:])

                # energy block 10 = (block 01)^T: stage 01 to SBUF, PE
                # transpose (f32) straight into pe2's first half.
                s01 = small.tile([P, P], f32, tag="s01")
                nc.scalar.copy(s01[:], pe1[:, ts(1, P)])

                def softmax(m):
                    """exp(mu - e)/Z in fp16 for row-block m (ACT/DVE)."""
                    pe = (pe1, pe2)[m]
                    mu = small.tile([P, 1], f32, tag="mu")
                    nc.vector.tensor_reduce(
                        mu[:], pe[:], axis=mybir.AxisListType.X,
                        op=mybir.AluOpType.min,
                    )
                    Pm = small.tile([P, C], f32, tag="Pm")
                    nc.scalar.activation(
                        Pm[:],
                        pe[:],
                        mybir.ActivationFunctionType.Exp,
                        bias=mu[:],
                        scale=-1.0,
                        accum_out=Zs[:, m : m + 1],
                    )
                    nc.vector.reciprocal(rZ[:, m : m + 1], Zs[:, m : m + 1])
                    Pm2 = small.tile([P, C], f16, tag=f"Pm2{m}", name="Pm2")
                    nc.vector.tensor_scalar_mul(Pm2[:], Pm[:], rZ[:, m : m + 1])
                    return Pm2

                def build_At(m, Pm2):
                    # At[m][:, k2, :] = gamma * (P/Z)^T (+ I on diagonal)
                    ptf = psum_x.tile([P, 512], f16, tag="px", name="ptf")
                    pt = ptf[:, :C]
                    for k2 in range(2):
                        nc.tensor.transpose(
                            pt[:, ts(k2, P)], Pm2[:, ts(k2, P)], idn[:]
                        )
                    nc.scalar.mul(At[m][:, :, :], pt[:], gv)
                    nc.vector.tensor_add(
                        At[m][:, m, :], At[m][:, m, :], idn[:]
                    )

                # m0's softmax resolves on ACT/DVE while the PE runs the
                # previous batch's matmul2
                Pm2_0 = softmax(0)

                if pending is not None:
                    run_mm2(*pending)
                    pending = None

                nc.tensor.transpose(pe2[:, ts(0, P)], s01[:], idn32)
                build_At(0, Pm2_0)
                Pm2_1 = softmax(1)
                if b == NB - 1:
                    # m1's softmax resolves under mm2(b, m0); only the m1
                    # half of the final matmul2 is exposed at the tail
                    run_mm2_m(b, At, xn, 0, 4)
                    build_At(1, Pm2_1)
                    run_mm2_m(b, At, xn, 1, 4)
                else:
                    build_At(1, Pm2_1)
                    pending = (b, At, xn)

    nc.compile()
    return nc


def _get_nc():
    if "nc" not in _CACHE:
        _CACHE["nc"] = _build_nc()
    return _CACHE["nc"]


def _make_aux(gamma_val):
    aux = np.zeros((P, 132), dtype=np.float32)
    aux[:, 0] = gamma_val
    aux[:, 4:132] = np.eye(P, dtype=np.float32)
    return aux


def kernel(x, gamma, _trace=False):
    import concourse.bass_utils as bass_utils

    x = np.ascontiguousarray(np.asarray(x, dtype=np.float32))
    gamma = np.asarray(gamma, dtype=np.float32).reshape(-1)

    nc = _get_nc()

    aux = _make_aux(gamma[0])
    idn = np.eye(P, dtype=np.float16)
    # xt[b, p, k, c] = x[b, c, k*128+p]
    xt_all = (
        x.astype(np.float16)
        .reshape(B, C, KT, P)
        .transpose(0, 3, 2, 1)
    )
    xnh_all = (
        x[:, :, T // 2 :].astype(np.float16).reshape(B, 2, P, T // 2)
    )
    in_maps = []
    for d in range(NCORES):
        in_maps.append(
            {
                "xt": np.ascontiguousarray(xt_all[d * NB : (d + 1) * NB]),
                "xnh": np.ascontiguousarray(xnh_all[d * NB : (d + 1) * NB]),
                "aux": aux,
                "idn": idn,
            }
        )

    res = bass_utils.run_bass_kernel_spmd(
        nc, in_maps, core_ids=list(range(NCORES)), trace=_trace
    )
    out = np.concatenate([r["o"] for r in res.results], axis=0)
    if _trace:
        _CACHE["last_results"] = res
    return out


# revision 44
# speedup vs baseline: 1.0240x; 1.0240x over previous
"""Trainium2 Bass kernel for nn_Attention_Module (sparse_attention).

Computation per batch b (x_b: [C=256, T=4096] fp32):
    energy = x_b @ x_b^T                      # (256, 256), K=4096
    attn   = softmax(rowmax(energy) - energy) # == exp(mu - e)/Z, mu = rowmin
    out    = gamma * (attn @ x_b) + x_b

Strategy (8 cores, pure data-parallel, 4 batches/core):
  - x is loaded from HBM exactly ONCE, as fp16 in t-major layout
    (xt[b, p, k, c] = x[b, c, k*128+p]) with 8KB/partition DMA lines
    (1 descriptor per partition -> cheap HWDGE issue). The c-major copy
    needed by matmul2 (xn) is derived on-chip with PE transposes.
    DMA drops from 41.9 MB/core (baseline) to 25.2 MB/core.
  - Energy is symmetric: only blocks 00/01/11 are computed; block 10 is
    a PE transpose of block 01 (saves 1/4 of matmul1).
  - matmul1 (both row-block chains) and the xn transposes are
    interleaved per k-tile so the PE tracks DMA arrival during the
    pipeline fill instead of stalling on whole tiles.
  - B = gamma*diag(1/Z)*P + I is materialized directly as the matmul2
    weights: P rows are scaled by 1/Z before the PE transpose, gamma is
    applied during the PSUM->SBUF copy of P^T, and the +x residual is
    the exact identity diagonal. matmul2 yields final output values, so
    PSUM->SBUF drains are plain copies (round-robined ACT/DVE).
  - Software pipeline: slot b = loads(b+1), mm1+xnT(b), mm2(b-1)
    (hides b's softmax latency), then softmax/At(b).
"""

import numpy as np

B, C, T = 32, 256, 4096
NCORES = 8
NB = B // NCORES  # batches per core
P = 128
KT = T // P  # 32 t-tiles of 128
KH = KT // 2  # 16 per half (xn is built as two half-tiles)
TC = T // 512  # 8 output chunks per m-block

_CACHE = {}


def _build_nc(variant=None):
    variant = variant or {}
    from contextlib import ExitStack

    import concourse.bacc as bacc
    import concourse.bass as bass
    import concourse.tile as tile
    from concourse import mybir

    f32 = mybir.dt.float32
    f16 = mybir.dt.float16
    ts = bass.ts

    nc = bacc.Bacc(
        "TRN2",
        target_bir_lowering=False,
        debug=False,
        enable_asserts=False,
        num_devices=NCORES,
    )

    # xt[b, p, k, c] = x[b, c, k*128+p]; per-partition lines are 8KB.
    xt_h = nc.dram_tensor("xt", [NB, P, KT, C], f16, kind="ExternalInput")
    # xnh[b, k2, p, t'] = x[b, k2*128+p, 1024+t']: upper 3/4 of the
    # c-major copy, loaded directly so only a quarter of xn needs PE
    # transposes (halves the ACT/DVE copy queues that gate the PE).
    xnh_h = nc.dram_tensor("xnh", [NB, 2, P, 3 * T // 4], f16, kind="ExternalInput")
    # aux: col0 = gamma (f32), cols 4:132 = identity (f32)
    aux_h = nc.dram_tensor("aux", [P, 132], f32, kind="ExternalInput")
    idn_h = nc.dram_tensor("idn", [P, P], f16, kind="ExternalInput")
    o_h = nc.dram_tensor("o", [NB, C, T], f32, kind="ExternalOutput")

    with tile.TileContext(nc) as tc:
        with ExitStack() as ctx:
            singles = ctx.enter_context(tc.tile_pool(name="singles", bufs=1))
            xt_pool = ctx.enter_context(tc.tile_pool(name="xt", bufs=2))
            xn_pool = ctx.enter_context(tc.tile_pool(name="xn", bufs=3))
            out_pool = ctx.enter_context(tc.tile_pool(name="out", bufs=3))
            att_pool = ctx.enter_context(tc.tile_pool(name="att", bufs=2))
            small = ctx.enter_context(tc.tile_pool(name="small", bufs=3))
            psum_e = ctx.enter_context(
                tc.tile_pool(name="psum_e", bufs=1, space="PSUM")
            )
            psum_x = ctx.enter_context(
                tc.tile_pool(name="psum_x", bufs=2, space="PSUM")
            )
            psum_o = ctx.enter_context(
                tc.tile_pool(name="psum_o", bufs=4, space="PSUM")
            )

            xt_ap = xt_h.ap()
            xnh_ap = xnh_h.ap()
            o_ap = o_h.ap()

            aux = singles.tile([P, 132], f32)
            nc.scalar.dma_start(aux[:], aux_h.ap())
            idn = singles.tile([P, P], f16)
            nc.scalar.dma_start(idn[:], idn_h.ap())
            gv = aux[:, 0:1]
            idn32 = aux[:, 4:132]

            def issue_loads(b):
                if b == 0:
                    # eight separate tiles: mm1 starts after 256KB lands,
                    # and arrivals pace the PE through the whole fill
                    KQ = KT // 8
                    qs = []
                    for q in range(8):
                        t_ = xt_pool.tile(
                            [P, KQ, C], f16, tag=f"xq{q}", name=f"xq{q}"
                        )
                        nc.sync.dma_start(
                            t_[:], xt_ap[b, :, q * KQ : (q + 1) * KQ, :]
                        )
                        qs.append(t_)
                    return (qs, KT // 8)
                xta = xt_pool.tile([P, KH, C], f16, tag="xta", name="xta")
                xtb = xt_pool.tile([P, KH, C], f16, tag="xtb", name="xtb")
                nc.sync.dma_start(xta[:], xt_ap[b, :, :KH, :])
                nc.sync.dma_start(xtb[:], xt_ap[b, :, KH:, :])
                return ([xta, xtb], KH)

            # round-robin copy engines for PSUM->SBUF drains
            cp_engines = [
                lambda o, i: nc.scalar.copy(o, i),
                lambda o, i: nc.vector.tensor_copy(o, i),
            ]
            cp_idx = [0]

            def copy_eng():
                e = cp_engines[cp_idx[0] % len(cp_engines)]
                cp_idx[0] += 1
                return e

            def run_mm2_m(pb, pAt, pxn, m, nsplit):
                """out(pb) row-block m = B^T @ x (final values) + stores."""
                TH = T // 2
                ot = out_pool.tile([P, T], f32, tag="ot", name="ot")
                for t8 in range(TC):  # [P,512] chunks, 4-deep psum ring
                    po = psum_o.tile([P, 512], f32)
                    src_x = pxn[0] if t8 < 2 else pxn[1]
                    tf = t8 * 512 if t8 < 2 else (t8 - 2) * 512
                    for k in range(2):
                        nc.tensor.matmul(
                            po[:],
                            lhsT=pAt[m][:, k, :],
                            rhs=src_x[:, k, tf : tf + 512],
                            start=(k == 0),
                            stop=(k == 1),
                        )
                    copy_eng()(ot[:, ts(t8, 512)], po[:])
                for sh in range(nsplit):
                    nc.sync.dma_start(
                        o_ap[pb].rearrange("(m p) t -> p m t", p=P)[
                            :, m, ts(sh, T // nsplit)
                        ],
                        ot[:, ts(sh, T // nsplit)],
                    )

            def run_mm2(pb, pAt, pxn):
                for m in range(2):
                    run_mm2_m(pb, pAt, pxn, m, 2)

            tiles = {0: issue_loads(0)}
            pending = None  # (b, At, xn) awaiting matmul2

            for b in range(NB):
                xtiles, kdiv = tiles.pop(b)
                if b + 1 < NB:
                    tiles[b + 1] = issue_loads(b + 1)

                xn = [
                    xn_pool.tile([P, 2, T // 4], f16, tag="xna", name="xna"),
                    xn_pool.tile(
                        [P, 2, 3 * T // 4], f16, tag="xnb", name="xnb"
                    ),
                ]
                nc.sync.dma_start(
                    xn[1][:], xnh_ap[b].rearrange("k p t -> p k t")
                )
                At = [
                    att_pool.tile([P, 2, P], f16, tag="Ata", name="Ata"),
                    att_pool.tile([P, 2, P], f16, tag="Atb", name="Atb"),
                ]
                Zs = small.tile([P, 2], f32, tag="Zs")
                rZ = small.tile([P, 2], f32, tag="rZ")

                # ---- interleaved mm1 (blocks 00/01 + block 11) and xn
                # transposes, per k-tile, tracking DMA arrival ----
                pe1 = psum_e.tile([P, C], f32, tag="pe1", name="pe1")
                pe2 = psum_e.tile([P, C], f32, tag="pe2", name="pe2")
                for k in range(KT):
                    src = xtiles[k // kdiv]
                    kk = k % kdiv
                    nc.tensor.matmul(
                        pe1[:],
                        lhsT=src[:, kk, ts(0, P)],
                        rhs=src[:, kk, :],
                        start=(k == 0),
                        stop=(k == KT - 1),
                    )
                    nc.tensor.matmul(
                        pe2[:, ts(1, P)],
                        lhsT=src[:, kk, ts(1, P)],
                        rhs=src[:, kk, ts(1, P)],
                        start=(k == 0),
                        stop=(k == KT - 1),
                    )
                    if k % 4 == 3 and k < KH // 2:
                        g4 = k - 3
                        half = 0
                        tb = g4 * P
                        for cb in range(2):
                            px = psum_x.tile([P, 512], f16, tag="px", name="px")
                            for j in range(4):
                                kg = g4 + j
                                nc.tensor.transpose(
                                    px[:, ts(j, P)],
                                    xtiles[kg // kdiv][:, kg % kdiv, ts(cb, P)],
                                    idn[:],
                                )
                            copy_eng()(xn[half][:, cb, tb : tb + 512], px[:])

                # energy block 10 = (block 01)^T: stage 01 to SBUF, PE
                # transpose (f32) straight into pe2's first half.
                s01 = small.tile([P, P], f32, tag="s01")
                nc.scalar.copy(s01[:], pe1[:, ts(1, P)])

                def softmax(m):
                    """exp(mu - e)/Z in fp16 for row-block m (ACT/DVE)."""
                    pe = (pe1, pe2)[m]
                    mu = small.tile([P, 1], f32, tag="mu")
                    nc.vector.tensor_reduce(
                        mu[:], pe[:], axis=mybir.AxisListType.X,
                        op=mybir.AluOpType.min,
                    )
                    Pm = small.tile([P, C], f32, tag="Pm")
                    nc.scalar.activation(
                        Pm[:],
                        pe[:],
                        mybir.ActivationFunctionType.Exp,
                        bias=mu[:],
                        scale=-1.0,
                        accum_out=Zs[:, m : m + 1],
                    )
                    nc.vector.reciprocal(rZ[:, m : m + 1], Zs[:, m : m + 1])
                    Pm2 = small.tile([P, C], f16, tag=f"Pm2{m}", name="Pm2")
                    nc.vector.tensor_scalar_mul(Pm2[:], Pm[:], rZ[:, m : m + 1])
                    return Pm2

                def build_At(m, Pm2):
                    # At[m][:, k2, :] = gamma * (P/Z)^T (+ I on diagonal)
                    ptf = psum_x.tile([P, 512], f16, tag="px", name="ptf")
                    pt = ptf[:, :C]
                    for k2 in range(2):
                        nc.tensor.transpose(
                            pt[:, ts(k2, P)], Pm2[:, ts(k2, P)], idn[:]
                        )
                    nc.scalar.mul(At[m][:, :, :], pt[:], gv)
                    nc.vector.tensor_add(
                        At[m][:, m, :], At[m][:, m, :], idn[:]
                    )

                # m0's softmax resolves on ACT/DVE while the PE runs the
                # previous batch's matmul2
                Pm2_0 = softmax(0)

                if pending is not None:
                    run_mm2(*pending)
                    pending = None

                nc.tensor.transpose(pe2[:, ts(0, P)], s01[:], idn32)
                build_At(0, Pm2_0)
                Pm2_1 = softmax(1)
                if b == NB - 1:
                    # m1's softmax resolves under mm2(b, m0); only the m1
                    # half of the final matmul2 is exposed at the tail
                    run_mm2_m(b, At, xn, 0, 4)
                    build_At(1, Pm2_1)
                    run_mm2_m(b, At, xn, 1, 4)
                else:
                    build_At(1, Pm2_1)
                    pending = (b, At, xn)

    nc.compile()
    return nc


def _get_nc():
    if "nc" not in _CACHE:
        _CACHE["nc"] = _build_nc()
    return _CACHE["nc"]


def _make_aux(gamma_val):
    aux = np.zeros((P, 132), dtype=np.float32)
    aux[:, 0] = gamma_val
    aux[:, 4:132] = np.eye(P, dtype=np.float32)
    return aux


def kernel(x, gamma, _trace=False):
    import concourse.bass_utils as bass_utils

    x = np.ascontiguousarray(np.asarray(x, dtype=np.float32))
    gamma = np.asarray(gamma, dtype=np.float32).reshape(-1)

    nc = _get_nc()

    aux = _make_aux(gamma[0])
    idn = np.eye(P, dtype=np.float16)
    # xt[b, p, k, c] = x[b, c, k*128+p]
    xt_all = (
        x.astype(np.float16)
        .reshape(B, C, KT, P)
        .transpose(0, 3, 2, 1)
    )
    xnh_all = (
        x[:, :, T // 4 :].astype(np.float16).reshape(B, 2, P, 3 * T // 4)
    )
    in_maps = []
    for d in range(NCORES):
        in_maps.append(
            {
                "xt": np.ascontiguousarray(xt_all[d * NB : (d + 1) * NB]),
                "xnh": np.ascontiguousarray(xnh_all[d * NB : (d + 1) * NB]),
                "aux": aux,
                "idn": idn,
            }
        )

    res = bass_utils.run_bass_kernel_spmd(
        nc, in_maps, core_ids=list(range(NCORES)), trace=_trace
    )
    out = np.concatenate([r["o"] for r in res.results], axis=0)
    if _trace:
        _CACHE["last_results"] = res
    return out
